# revision 3
# baseline (speedup 1.0000x reference)
"""Bootstrap-ensemble MLP (100 models, D=16 -> H=128 x5 -> mu/sigma heads)
on 8 Trainium2 NeuronCores.

Sharding: every core runs an identical SPMD program over 25 models x 8192
batch points (model axis split 4 ways x batch split 2 ways) -- perfectly
balanced.  All per-core weights are pre-arranged on the host into the exact
SBUF layouts the TensorEngine wants (lhsT = pre-transposed stationary
operand), so the device does no transposes at all.

Per (model, 1024-point chunk): 6 "layers" x 2 matmuls of N=512 on PE;
bias+ReLU fused into single ScalarE activation or VectorE tensor_scalar
ops (split across both engines, since the elementwise epilogue is the
co-bottleneck with PE); mu/sigma heads of all 25 models accumulate into a
single [50, 1024] PSUM tile via zero-padded head weights, finished with
Copy/Exp activations with the bias folded in.
"""

import os

import numpy as np

M = 100  # n_models
D = 16  # input_dim
H = 128  # hidden_dim
O = 1  # output_dim
NH = 4  # n_hidden
N = 16384  # batch of query points

NCORES = 8
MPC = 25  # models per core
NB = 4  # model blocks
NHALF = N // 2  # 8192 points per core
CH = 1024  # chunk of batch points processed at once
NCH = NHALF // CH  # 8 chunks
MM_N = 512  # fp32 matmul moving free dim (one PSUM bank)

_CACHE: dict = {}


def _build_module():
    import concourse.bacc as bacc
    import concourse.mybir as mybir
    import concourse.tile as tile

    f32 = mybir.dt.float32
    AF = mybir.ActivationFunctionType
    ALU = mybir.AluOpType

    nc = bacc.Bacc(
        "TRN2",
        target_bir_lowering=False,
        debug=False,
        num_devices=NCORES,
    )

    xt_d = nc.dram_tensor("xt", [128, NHALF], f32, kind="ExternalInput")
    w1t_d = nc.dram_tensor("w1t", [D, MPC * H], f32, kind="ExternalInput")
    wht_d = nc.dram_tensor("wht", [H, MPC * NH * H], f32, kind="ExternalInput")
    whd_d = nc.dram_tensor("whd", [H, MPC * 64], f32, kind="ExternalInput")
    b1_d = nc.dram_tensor("b1", [H, MPC], f32, kind="ExternalInput")
    bh_d = nc.dram_tensor("bh", [H, MPC * NH], f32, kind="ExternalInput")
    bhd_d = nc.dram_tensor("bhd", [64, 1], f32, kind="ExternalInput")
    mu_d = nc.dram_tensor("mu", [MPC, NHALF], f32, kind="ExternalOutput")
    sig_d = nc.dram_tensor("sig", [MPC, NHALF], f32, kind="ExternalOutput")

    with tile.TileContext(nc) as tc:
        with (
            tc.tile_pool(name="const", bufs=1) as const,
            tc.tile_pool(name="hpool", bufs=6) as hpool,
            tc.tile_pool(name="opool", bufs=2) as opool,
            tc.tile_pool(name="mmpsum", bufs=3, space="PSUM") as mmpsum,
            tc.tile_pool(name="hdpsum", bufs=1, space="PSUM") as hdpsum,
        ):
            xt = const.tile([128, NHALF], f32)
            w1t = const.tile([D, MPC * H], f32)
            wht = const.tile([H, MPC * NH * H], f32)
            whd = const.tile([H, MPC * 64], f32)
            b1 = const.tile([H, MPC], f32)
            bh = const.tile([H, MPC * NH], f32)
            bhd = const.tile([64, 1], f32)

            nc.sync.dma_start(w1t[:], w1t_d[:])
            nc.sync.dma_start(b1[:], b1_d[:])
            nc.sync.dma_start(bh[:], bh_d[:])
            nc.sync.dma_start(bhd[:], bhd_d[:])
            nc.sync.dma_start(whd[:], whd_d[:])
            # chunked so the first models' matmuls don't wait on the full blob
            for m in range(MPC):
                s = m * NH * H
                nc.sync.dma_start(wht[:, s : s + NH * H], wht_d[:, s : s + NH * H])
            for nt in range(NCH):
                s = nt * CH
                nc.sync.dma_start(xt[:, s : s + CH], xt_d[:, s : s + CH])

            def relu_act(dst, src, bias_ap):
                nc.scalar.activation(dst, src, AF.Relu, bias=bias_ap)

            def relu_dve(dst, src, bias_ap):
                nc.vector.tensor_scalar(dst, src, bias_ap, 0.0, ALU.add, ALU.max)

            for nt in range(NCH):
                c0 = nt * CH
                hp = hdpsum.tile([64, CH], f32, tag="hp")
                for m in range(MPC):
                    # layer 1: [16,128] weights vs xT chunk
                    ps = mmpsum.tile([128, CH], f32, tag="mm")
                    lhs1 = w1t[:, m * H : (m + 1) * H]
                    for s in range(0, CH, MM_N):
                        nc.tensor.matmul(
                            ps[:, s : s + MM_N],
                            lhs1,
                            xt[0:D, c0 + s : c0 + s + MM_N],
                            start=True,
                            stop=True,
                        )
                    h = hpool.tile([128, CH], f32, tag="h")
                    # split the bias+relu across ACT and DVE halves
                    relu_act(h[:, 0:MM_N], ps[:, 0:MM_N], b1[:, m : m + 1])
                    relu_dve(h[:, MM_N:CH], ps[:, MM_N:CH], b1[:, m : m + 1])
                    # hidden layers
                    for i in range(NH):
                        ps = mmpsum.tile([128, CH], f32, tag="mm")
                        lhsh = wht[:, (m * NH + i) * H : (m * NH + i + 1) * H]
                        for s in range(0, CH, MM_N):
                            nc.tensor.matmul(
                                ps[:, s : s + MM_N],
                                lhsh,
                                h[:, s : s + MM_N],
                                start=True,
                                stop=True,
                            )
                        hn = hpool.tile([128, CH], f32, tag="h")
                        bias_ap = bh[:, m * NH + i : m * NH + i + 1]
                        if i == NH - 1:
                            # 75/25 split to balance ACT vs DVE totals
                            relu_act(hn[:, 0:768], ps[:, 0:768], bias_ap)
                            relu_dve(hn[:, 768:CH], ps[:, 768:CH], bias_ap)
                        elif i % 2 == 0:
                            relu_act(hn[:], ps[:], bias_ap)
                        else:
                            relu_dve(hn[:], ps[:], bias_ap)
                        h = hn
                    # heads: accumulate all 25 models into one [50, CH] psum
                    lhshd = whd[:, m * 64 : (m + 1) * 64]
                    for s in range(0, CH, MM_N):
                        nc.tensor.matmul(
                            hp[:, s : s + MM_N],
                            lhshd,
                            h[:, s : s + MM_N],
                            start=(m == 0),
                            stop=(m == MPC - 1),
                        )
                mu_t = opool.tile([MPC, CH], f32, tag="mu")
                sig_t = opool.tile([MPC, CH], f32, tag="sig")
                nc.scalar.activation(
                    mu_t[:], hp[0:MPC, :], AF.Identity, bias=bhd[0:MPC, :]
                )
                nc.scalar.activation(
                    sig_t[:], hp[32 : 32 + MPC, :], AF.Exp, bias=bhd[32 : 32 + MPC, :]
                )
                nc.sync.dma_start(mu_d[:, c0 : c0 + CH], mu_t[:])
                nc.sync.dma_start(sig_d[:, c0 : c0 + CH], sig_t[:])

    nc.compile()
    return nc


def _get_module():
    if "nc" not in _CACHE:
        _CACHE["nc"] = _build_module()
    return _CACHE["nc"]


def _shard_inputs(x, W1, b1, Wh, bh, Wmu, bmu, Wsig, bsig):
    """Build the per-core input maps (host-side layout prep)."""
    in_maps = []
    for c in range(NCORES):
        mb, half = c % NB, c // NB
        ms = slice(MPC * mb, MPC * (mb + 1))
        xh = x[NHALF * half : NHALF * (half + 1), :]  # [8192, 16]
        xtr = np.ascontiguousarray(xh.T)  # [16, 8192]
        xt_full = np.zeros((128, NHALF), dtype=np.float32)
        for rep in range(4):  # replicas at partition 0/32/64/96 (row-tiling ready)
            xt_full[32 * rep : 32 * rep + D, :] = xtr

        w1 = W1[ms]  # [25, 128, 16]
        w1t = np.ascontiguousarray(
            w1.transpose(2, 0, 1).reshape(D, MPC * H)
        )  # [16, 25*128], col m*H+h = W1[m,h,:]

        wh = Wh[ms]  # [25, 4, 128, 128] (out, in)
        wht = np.ascontiguousarray(
            wh.transpose(3, 0, 1, 2).reshape(H, MPC * NH * H)
        )  # [h_in, (m, i, h_out)]

        whd = np.zeros((H, MPC * 64), dtype=np.float32)
        for m in range(MPC):
            base = m * 64
            whd[:, base + m] = Wmu[ms][m, 0, :]
            whd[:, base + 32 + m] = Wsig[ms][m, 0, :]

        b1p = np.ascontiguousarray(b1[ms].T)  # [128, 25]
        bhp = np.ascontiguousarray(
            bh[ms].transpose(2, 0, 1).reshape(H, MPC * NH)
        )  # [128, (m, i)]
        bhdp = np.zeros((64, 1), dtype=np.float32)
        bhdp[0:MPC, 0] = bmu[ms][:, 0]
        bhdp[32 : 32 + MPC, 0] = bsig[ms][:, 0]

        in_maps.append(
            {
                "xt": xt_full,
                "w1t": w1t,
                "wht": wht,
                "whd": whd,
                "b1": b1p,
                "bh": bhp,
                "bhd": bhdp,
            }
        )
    return in_maps


def _run(in_maps, trace=False):
    from concourse.bass_utils import run_bass_kernel_spmd

    nc = _get_module()
    return run_bass_kernel_spmd(
        nc, in_maps, list(range(NCORES)), trace=trace
    )


def kernel(x, W1, b1, Wh, bh, Wmu, bmu, Wsig, bsig):
    args = [
        np.ascontiguousarray(np.asarray(a, dtype=np.float32))
        for a in (x, W1, b1, Wh, bh, Wmu, bmu, Wsig, bsig)
    ]
    in_maps = _shard_inputs(*args)
    res = _run(in_maps, trace=bool(int(os.environ.get("KERNEL_TRACE", "0"))))
    _CACHE["last_results"] = res

    mu = np.empty((M, N), dtype=np.float32)
    sig = np.empty((M, N), dtype=np.float32)
    for c in range(NCORES):
        mb, half = c % NB, c // NB
        ms = slice(MPC * mb, MPC * (mb + 1))
        ns = slice(NHALF * half, NHALF * (half + 1))
        mu[ms, ns] = res.results[c]["mu"]
        sig[ms, ns] = res.results[c]["sig"]
    return (mu.reshape(M, N, O), sig.reshape(M, N, O))
